# revision 3
# baseline (speedup 1.0000x reference)
"""BiLSTM-CRF Trainium kernel, v2: 8-way chunked recurrence.

Sharding (8-core SPMD, one program, per-core data):
 - cores 0-3: forward LSTM over token chunks [512j-64, 512j+512) (64-step
   warm-up, first 512-token chunk gets the exact h0/c0 via a masked state
   reset between loop iterations 3 and 4); cores 4-7: the same on the
   host-reversed token stream (backward direction).
 - weights ship as per-core quarters (k-chunk of [wihT; whhT], fp16) and are
   reassembled on-device with a subgroup AllGather ([[0..3],[4..7]]).
 - each core computes its 512-token half-feats (its direction's w_out part),
   transposes and indirect-DMA-scatters them into a global [2048, 12] DRAM
   buffer at host-computed rows (this also undoes the backward reversal),
   then one AllReduce(+) makes full feats visible everywhere.
 - CRF: each core gathers its 256-token block, composes the block transfer
   matrix sequentially (log-semiring [12,12] steps), AllGather + 8-step fold
   -> alpha; per-block gold emit partials ride along in the AllGather.
 - host adds the transition part of the gold score (pure host data).

Inputs per core: pk16 [128, 6656] f16, pk32 [128, 280] f32, pki [128, 8] i32
(~1.8 MB/core vs 13.8 MB for the unchunked kernel).

Gate order in the permuted layouts: i, f, o, g  (G index 0..3),
pre/act column cc = G*4 + q, h component kappa(p,q) = q*128 + p.
"""
import numpy as np
import concourse.bass as bass
import concourse.mybir as mybir
import concourse.tile as tile
from concourse.masks import make_identity

F32 = mybir.dt.float32
F16 = mybir.dt.float16
I32 = mybir.dt.int32
AF = mybir.ActivationFunctionType
OP = mybir.AluOpType
AX = mybir.AxisListType

S, V, E, HD, T = 2048, 50257, 512, 512, 12
NG = 4 * HD          # 2048 gate rows per direction
P = 128
L = 512              # output tokens per core
W = 64               # warm-up steps
N = L + W            # 576 LSTM steps per core
U_LSTM = 16
NITER = N // U_LSTM  # 36
RESET_IT = W // U_LSTM  # 4
BLK = S // 8         # 256 CRF tokens per core
U_CRF = 16
NEG = -1e6

OG = [0, 1, 3, 2]    # our gate G=[i,f,o,g] -> original block [i,f,g,o]

# pk16 column map
C_WQ, C_XT, C_OH = 0, 4096, 4096 + 4 * N
PK16_W = C_OH + BLK
# pk32 column map
C_GB, C_HI, C_CI, C_WO, C_MSK = 0, 16, 20, 24, 72
C_TR, C_EYE, C_BO, C_TKJ = 108, 120, 132, 136
C_TE = C_TKJ + 144
PK32_W = C_TE + 12


def split_multi_waits(nc) -> int:
    """Walrus builds accept at most one sync-wait/update per instruction:
    split extras onto NoOps on the same engine."""
    n_split = 0
    for f in nc.m.functions:
        for bb in f.blocks:
            insts = bb.instructions
            out = []
            changed = False
            for inst in insts:
                si = inst.sync_info
                if si is None:
                    out.append(inst)
                    continue
                waits = list(si.on_wait)
                updates = list(si.on_update)
                if len(waits) <= 1 and len(updates) <= 1:
                    out.append(inst)
                    continue
                changed = True
                eng = inst.engine
                pre = []
                for w in waits[:-1]:
                    nop = mybir.InstNoOp(
                        name=nc.get_next_instruction_name(), ins=[], outs=[]
                    )
                    nop.engine = eng
                    nop.sync_info = mybir.SyncInfo(on_wait=[w], on_update=[])
                    pre.append(nop)
                    n_split += 1
                post = []
                for u in updates[1:]:
                    nop = mybir.InstNoOp(
                        name=nc.get_next_instruction_name(), ins=[], outs=[]
                    )
                    nop.engine = eng
                    nop.sync_info = mybir.SyncInfo(on_wait=[], on_update=[u])
                    post.append(nop)
                    n_split += 1
                inst.sync_info = mybir.SyncInfo(
                    on_wait=waits[-1:], on_update=updates[:1]
                )
                out.extend(pre)
                out.append(inst)
                out.extend(post)
            if changed:
                bb.instructions = out
    return n_split


# ---------------------------------------------------------------- host prep

def perm_rec():
    n = np.arange(NG)
    j, rem = n // 512, n % 512
    cc, i = rem // 32, rem % 32
    G, q = cc // 4, cc % 4
    og = np.array(OG)[G]
    return og * 512 + q * 128 + 32 * j + i


def perm_in():
    n = np.arange(NG)
    m, pp = n // 128, n % 128
    G, q = m // 4, m % 4
    og = np.array(OG)[G]
    return og * 512 + q * 128 + pp


def w_chunked_T(Wp):
    """[NG, 512] f32 -> [128, 4*NG]: out[p, k*NG+n] = Wp[n, k*128+p]."""
    return np.ascontiguousarray(
        Wp.reshape(NG, 4, 128).transpose(2, 1, 0).reshape(P, 4 * NG))


def part_major(a, cols):
    """[rows, 128*cols]-style: [n, D] with D=128*cols -> [128, cols*n]?
    Here: vec arrangement [D] -> [128, D//128] with out[p, q] = a[q*128+p]."""
    return np.ascontiguousarray(a.reshape(cols, 128).T)


_PREP_CACHE = {}


def prep_all(inputs):
    """Build the 8 per-core in_maps. Returns (in_maps, gold_trans_scalar)."""
    sent = np.asarray(inputs["sentence"]).astype(np.int64).reshape(-1)
    gold = np.asarray(inputs["gold_tags"]).astype(np.int64).reshape(-1)
    emb = np.asarray(inputs["emb"], np.float32)
    trans = np.asarray(inputs["transitions"], np.float32)
    w_out = np.asarray(inputs["w_out"], np.float32)
    b_out = np.asarray(inputs["b_out"], np.float32)
    h0 = np.asarray(inputs["h0"], np.float32)
    c0 = np.asarray(inputs["c0"], np.float32)

    pr, pi = perm_rec(), perm_in()
    # per-direction weight packs [128, 4*NG] fp16 (input-proj + recurrent)
    dirw = []
    for d, (wih, whh, b) in enumerate((
        (inputs["wih_f"], inputs["whh_f"], inputs["b_f"]),
        (inputs["wih_b"], inputs["whh_b"], inputs["b_b"]),
    )):
        wih = np.asarray(wih, np.float32)
        whh = np.asarray(whh, np.float32)
        b = np.asarray(b, np.float32)
        dirw.append(dict(
            wihT=w_chunked_T(wih[pi]).astype(np.float16),
            whhT=w_chunked_T(whh[pr]).astype(np.float16),
            ginb=np.ascontiguousarray(b[pi].reshape(16, 128).T),
            h0p=part_major(h0[d], 4),
            c0p=part_major(c0[d], 4),
            woutT=np.ascontiguousarray(
                w_out[:, d * 512:(d + 1) * 512].reshape(T, 4, 128)
                .transpose(2, 1, 0).reshape(P, 48)),
        ))

    x_f = emb[sent]                       # [S, E] f32
    x_b = x_f[::-1]

    # gold transition score (host; exact)
    tags = np.concatenate([[0], gold])
    gold_trans = float(trans[tags[1:], tags[:-1]].sum()) + float(
        trans[1, tags[-1]])

    in_maps = []
    for core in range(8):
        d, j = core // 4, core % 4
        dw = dirw[d]
        x = x_f if d == 0 else x_b

        # x window [512j-64, 512j+512), clipped; partition-major fp16
        pos = np.clip(512 * j - W + np.arange(N), 0, S - 1)
        xs = x[pos]                                        # [N, 512]
        xT = np.ascontiguousarray(
            xs.reshape(N, 4, 128).transpose(2, 1, 0).reshape(P, 4 * N)
        ).astype(np.float16)

        pk16 = np.zeros((P, PK16_W), np.float16)
        pk16[:, C_WQ + 0 * NG: C_WQ + 1 * NG] = dw["wihT"][:, j * NG:(j + 1) * NG]
        pk16[:, C_WQ + 1 * NG: C_WQ + 2 * NG] = dw["whhT"][:, j * NG:(j + 1) * NG]
        pk16[:, C_XT: C_XT + 4 * N] = xT
        # gold one-hot for CRF block [256*core, 256*core+256)
        gb = gold[BLK * core: BLK * (core + 1)]
        oh = np.zeros((T, BLK), np.float16)
        oh[gb, np.arange(BLK)] = 1.0
        pk16[0:T, C_OH: C_OH + BLK] = oh

        pk32 = np.zeros((P, PK32_W), np.float32)
        pk32[:, C_GB: C_GB + 16] = dw["ginb"]
        if j == 0:
            pk32[:, C_HI: C_HI + 4] = dw["h0p"]
            pk32[:, C_CI: C_CI + 4] = dw["c0p"]
        pk32[:, C_WO: C_WO + 48] = dw["woutT"]
        msk = np.ones((P, NITER), np.float32)
        if j == 0:
            msk[:, RESET_IT] = 0.0
        pk32[:, C_MSK: C_MSK + NITER] = msk
        pk32[0:T, C_TR: C_TR + T] = trans
        pk32[0:T, C_EYE: C_EYE + T] = np.where(
            np.eye(T, dtype=bool), 0.0, NEG).astype(np.float32)
        pk32[0:T, C_BO: C_BO + 1] = b_out.reshape(T, 1)
        pk32[0:1, C_TKJ: C_TKJ + 144] = np.ascontiguousarray(
            trans.T).reshape(1, 144)
        pk32[0:1, C_TE: C_TE + 12] = trans[1:2, :]

        pki = np.zeros((P, 8), np.int32)
        # feats scatter rows: local output col 128*kk+p -> global token row
        for kk in range(4):
            t_local = 512 * j + 128 * kk + np.arange(128)
            g_row = t_local if d == 0 else (S - 1) - t_local
            pki[:, kk] = g_row
        # CRF block gather rows
        for t in range(2):
            pki[:, 4 + t] = BLK * core + 128 * t + np.arange(128)

        in_maps.append(dict(pk16=pk16, pk32=pk32, pki=pki))
    return in_maps, gold_trans


# ---------------------------------------------------------------- device code

def build(debug=0, upto=99):
    """upto: 1=unpack+wAG, 2=+gin, 3=+LSTM, 4=+feats/AllReduce, 5=+CRF
    compose, 99=full. Truncated builds end with a dummy out write."""
    nc = bass.Bass("TRN2", target_bir_lowering=False, debug=False,
                   num_devices=8)

    pk16 = nc.dram_tensor("pk16", [P, PK16_W], F16, kind="ExternalInput")
    pk32 = nc.dram_tensor("pk32", [P, PK32_W], F32, kind="ExternalInput")
    pki = nc.dram_tensor("pki", [P, 8], I32, kind="ExternalInput")
    out_d = nc.dram_tensor("out", [1, 1], F32, kind="ExternalOutput")
    if debug:
        hdbg_d = nc.dram_tensor("hdbg", [P, 4 * N], F16, kind="ExternalOutput")
        fdbg_d = nc.dram_tensor("fdbg", [S, T], F32, kind="ExternalOutput")
        bdbg_d = nc.dram_tensor("bdbg", [T, BLK], F32, kind="ExternalOutput")
        adbg_d = nc.dram_tensor("adbg", [T, T], F32, kind="ExternalOutput")

    with tile.TileContext(nc) as tc:
        with (
            tc.tile_pool(name="sb", bufs=1) as sb,
            tc.tile_pool(name="ps", bufs=1, space="PSUM") as ps,
            tc.tile_pool(name="dr", bufs=1, space="DRAM") as dr,
        ):
            # ---------------- unpack + weight AllGather
            pk32_sb = sb.tile([P, PK32_W], F32, name="pk32_sb")
            nc.sync.dma_start(pk32_sb[:], pk32.ap())
            pki_sb = sb.tile([P, 8], I32, name="pki_sb")
            nc.sync.dma_start(pki_sb[:], pki.ap())
            xt_h = sb.tile([P, 4 * N], F16, name="xt_h")
            nc.sync.dma_start(xt_h[:], pk16.ap()[:, C_XT: C_XT + 4 * N])
            oneh = sb.tile([T, BLK], F16, name="oneh")
            nc.sync.dma_start(oneh[:], pk16.ap()[0:T, C_OH: C_OH + BLK])

            wst = sb.tile([P, 4096], F16, name="wst")
            nc.sync.dma_start(wst[:], pk16.ap()[:, C_WQ: C_WQ + 4096])
            wag_in = dr.tile([P, 4096], F16, name="wag_in")
            wcat = dr.tile([4 * P, 4096], F16, name="wcat")
            nc.sync.dma_start(wag_in[:], wst[:])
            nc.gpsimd.collective_compute(
                "AllGather", OP.bypass,
                replica_groups=[[0, 1, 2, 3], [4, 5, 6, 7]],
                ins=[wag_in[:].opt()], outs=[wcat[:].opt()],
            )
            wih_h = sb.tile([P, 4 * NG], F16, name="wih_h")
            whh_h = sb.tile([P, 4 * NG], F16, name="whh_h")
            for k in range(4):
                nc.sync.dma_start(
                    wih_h[:, k * NG:(k + 1) * NG],
                    wcat[:][128 * k:128 * (k + 1), 0:NG])
                nc.sync.dma_start(
                    whh_h[:, k * NG:(k + 1) * NG],
                    wcat[:][128 * k:128 * (k + 1), NG:2 * NG])

            ident = sb.tile([P, P], F32, name="ident")
            make_identity(nc, ident[:])

            def _trunc(src_ap):
                t_ = sb.tile([1, 1], F32, name="trunc")
                nc.vector.tensor_copy(t_[:], src_ap)
                nc.sync.dma_start(out_d.ap(), t_[:])

            if upto <= 1:
                _trunc(wih_h[0:1, 0:1])
                return nc

            gin_b = pk32_sb[:, C_GB: C_GB + 16]
            hI = pk32_sb[:, C_HI: C_HI + 4]
            cI = pk32_sb[:, C_CI: C_CI + 4]
            msk = pk32_sb[:, C_MSK: C_MSK + NITER]
            trans_sb = pk32_sb[0:T, C_TR: C_TR + T]
            eyelog = pk32_sb[0:T, C_EYE: C_EYE + T]
            bout = pk32_sb[0:T, C_BO: C_BO + 1]
            tkj_sb = pk32_sb[0:1, C_TKJ: C_TKJ + 144]
            tend_sb = pk32_sb[0:1, C_TE: C_TE + 12]

            wout_h = sb.tile([P, 48], F16, name="wout_h")
            nc.vector.tensor_copy(wout_h[:], pk32_sb[:, C_WO: C_WO + 48])
            hI_h = sb.tile([P, 4], F16, name="hI_h")
            nc.vector.tensor_copy(hI_h[:], hI)

            # ---------------- input projection: gin[p, u*16+m]
            gin_sb = sb.tile([P, 16 * N], F16, name="gin_sb")
            gin_tm = gin_sb[:].rearrange("p (t m) -> p t m", m=16)
            pp_tag = dict(tag="pp", bufs=2)
            for m in range(16):
                for s0, sl in ((0, 512), (512, N - 512)):
                    pp = ps.tile([P, 512], F32, name="pp", **pp_tag)
                    for k in range(4):
                        nc.tensor.matmul(
                            out=pp[:, 0:sl],
                            lhsT=wih_h[:, k * NG + m * P: k * NG + (m + 1) * P],
                            rhs=xt_h[:, k * N + s0: k * N + s0 + sl],
                            start=(k == 0), stop=(k == 3),
                        )
                    nc.vector.tensor_tensor(
                        out=gin_tm[:, s0:s0 + sl, m:m + 1],
                        in0=pp[:, 0:sl].rearrange("p (t o) -> p t o", o=1),
                        in1=gin_b[:, m:m + 1].to_broadcast([P, sl]).rearrange(
                            "p (t o) -> p t o", o=1),
                        op=OP.add,
                    )

            if upto <= 2:
                _trunc(gin_sb[0:1, 0:1])
                return nc

            # ---------------- LSTM: N steps, masked state reset at it=4
            H_h = sb.tile([P, 4 * N + 4], F16, name="H_h")
            hstage = sb.tile([P, 4 * U_LSTM], F16, name="hstage")
            nc.vector.memset(hstage[:], 0.0)
            c_sb = sb.tile([P, 4], F32, name="c_sb")
            nc.vector.memset(c_sb[:], 0.0)
            g_ps = ps.tile([P, 512], F32, name="g_ps", tag="g")
            nc.vector.memset(g_ps[:], 0.0)
            gt_sb = sb.tile([P, 512], F32, name="gt_sb")
            pre_sb = sb.tile([P, 16], F32, name="pre_sb")
            act_sb = sb.tile([P, 16], F32, name="act_sb")
            z_sb = sb.tile([P, 4], F32, name="z_sb")
            fc_sb = sb.tile([P, 4], F32, name="fc_sb")
            tc_sb = sb.tile([P, 4], F32, name="tc_sb")
            th_sb = sb.tile([P, 4], F16, name="th_sb")
            tc4_sb = sb.tile([P, 4], F32, name="tc4_sb")
            gstage = sb.tile([P, 16 * U_LSTM], F16, name="gstage")

            _gt = gt_sb[:]
            gt_strided = bass.AP(_gt.tensor, _gt.offset, [_gt.ap[0], [32, 16]])

            def lstm_step(u):
                up = (u - 1) % U_LSTM
                for k in range(4):
                    lcol = hstage[:, 4 * up + k:4 * up + k + 1]
                    for jj in range(4):
                        nc.tensor.matmul(
                            out=g_ps[32 * jj:32 * jj + 1, :],
                            lhsT=lcol,
                            rhs=whh_h[:, k * NG + jj * 512: k * NG + (jj + 1) * 512],
                            start=(k == 0), stop=(k == 3),
                            tile_position=(0, 32 * jj),
                        )
                nc.vector.transpose(gt_sb[:], g_ps[:])
                nc.vector.tensor_tensor(
                    out=pre_sb[:], in0=gt_strided,
                    in1=gstage[:, 16 * u:16 * (u + 1)], op=OP.add,
                )
                nc.scalar.activation(act_sb[:, 0:12], pre_sb[:, 0:12], AF.Sigmoid)
                nc.scalar.activation(act_sb[:, 12:16], pre_sb[:, 12:16], AF.Tanh)
                nc.vector.tensor_tensor(
                    out=z_sb[:], in0=act_sb[:, 0:4], in1=act_sb[:, 12:16],
                    op=OP.mult)
                nc.vector.tensor_tensor(
                    out=fc_sb[:], in0=act_sb[:, 4:8], in1=c_sb[:], op=OP.mult)
                nc.vector.tensor_tensor(
                    out=c_sb[:], in0=fc_sb[:], in1=z_sb[:], op=OP.add)
                nc.scalar.activation(tc_sb[:], c_sb[:], AF.Tanh)
                nc.vector.tensor_tensor(
                    out=hstage[:, 4 * u:4 * (u + 1)], in0=act_sb[:, 8:12],
                    in1=tc_sb[:], op=OP.mult)

            with tc.For_i(0, NITER, hint_engines=(
                    mybir.EngineType.PE, mybir.EngineType.DVE,
                    mybir.EngineType.Activation)) as it:
                # masked state reset: h <- (h-hI)*m + hI, c likewise
                mcol = msk[:, bass.ds(it, 1)]
                nc.vector.tensor_tensor(
                    out=th_sb[:], in0=hstage[:, 60:64], in1=hI_h[:], op=OP.subtract)
                nc.vector.scalar_tensor_tensor(
                    out=hstage[:, 60:64], in0=th_sb[:], scalar=mcol,
                    in1=hI_h[:], op0=OP.mult, op1=OP.add)
                nc.vector.tensor_tensor(
                    out=tc4_sb[:], in0=c_sb[:], in1=cI, op=OP.subtract)
                nc.vector.scalar_tensor_tensor(
                    out=c_sb[:], in0=tc4_sb[:], scalar=mcol,
                    in1=cI, op0=OP.mult, op1=OP.add)
                nc.scalar.copy(gstage[:],
                               gin_sb[:, bass.ds(16 * U_LSTM * it, 16 * U_LSTM)])
                for u in range(U_LSTM):
                    lstm_step(u)
                nc.scalar.copy(H_h[:, bass.ds(4 * U_LSTM * it, 4 * U_LSTM)],
                               hstage[:])

            if debug:
                nc.sync.dma_start(hdbg_d.ap(), H_h[:, 0:4 * N])

            if upto <= 3:
                _trunc(H_h[0:1, 0:1])
                return nc

            # ---------------- feats: [12, 512] half-feats -> scatter -> AllReduce
            fp = ps.tile([T, 512], F32, name="fp", **pp_tag)
            Hwin = H_h[:, 4 * W: 4 * (W + L)].rearrange("p (t q) -> p t q", q=4)
            for q in range(4):
                nc.tensor.matmul(
                    out=fp[:], lhsT=wout_h[:, q * 12:(q + 1) * 12],
                    rhs=Hwin[:, :, q:q + 1], start=(q == 0), stop=(q == 3),
                )
            f_my = sb.tile([T, 512], F32, name="f_my")
            nc.vector.tensor_copy(f_my[:], fp[:])

            gfeats = dr.tile([S, T], F32, name="gfeats")
            zed = sb.tile([P, S * T // P], F32, name="zed")
            nc.vector.memset(zed[:], 0.0)
            nc.sync.dma_start(
                gfeats[:].rearrange("(p a) f -> p (a f)", p=P), zed[:])
            tp_tag = dict(tag="tp", bufs=2)
            for kk in range(4):
                tp = ps.tile([P, T], F32, name="tp", **tp_tag)
                nc.tensor.transpose(
                    out=tp[:], in_=f_my[:, P * kk: P * (kk + 1)],
                    identity=ident[0:T, 0:T])
                ft = sb.tile([P, T], F32, name="ft", tag="ft", bufs=2)
                nc.scalar.activation(ft[:], tp[:], AF.Copy)
                nc.gpsimd.indirect_dma_start(
                    out=gfeats[:], out_offset=bass.IndirectOffsetOnAxis(
                        ap=pki_sb[:, kk:kk + 1], axis=0),
                    in_=ft[:], in_offset=None,
                )
            gfall = dr.tile([S, T], F32, name="gfall")
            nc.gpsimd.collective_compute(
                "AllReduce", OP.add,
                replica_groups=[list(range(8))],
                ins=[gfeats[:].opt()], outs=[gfall[:].opt()],
            )
            if debug:
                nc.sync.dma_start(fdbg_d.ap(), gfall[:])

            # ---------------- CRF block: gather + compose
            f_blk = sb.tile([T, BLK], F32, name="f_blk")
            for t in range(2):
                fbp = sb.tile([P, T], F32, name="fbp", tag="ft", bufs=2)
                nc.gpsimd.indirect_dma_start(
                    out=fbp[:], out_offset=None, in_=gfall[:],
                    in_offset=bass.IndirectOffsetOnAxis(
                        ap=pki_sb[:, 4 + t:5 + t], axis=0),
                )
                tpc = ps.tile([T, P], F32, name="tpc", **tp_tag)
                nc.tensor.transpose(out=tpc[:], in_=fbp[:], identity=ident[:])
                nc.scalar.activation(
                    f_blk[:, P * t: P * (t + 1)], tpc[:], AF.Copy)
            nc.vector.tensor_scalar(
                out=f_blk[:], in0=f_blk[:], scalar1=bout[:, 0:1],
                scalar2=None, op0=OP.add)
            if debug:
                nc.sync.dma_start(bdbg_d.ap(), f_blk[:])

            if upto <= 4:
                _trunc(f_blk[0:1, 0:1])
                return nc

            # compose: A <- A o Step_u, descending u
            ones12 = sb.tile([1, T], F32, name="ones12")
            nc.vector.memset(ones12[:], 1.0)
            prow = sb.tile([32, 32], F32, name="prow")
            nc.vector.memset(prow[:], 0.0)
            nc.vector.memset(prow[0:1, 1:T], NEG)
            scr = sb.tile([32, 32], F32, name="scr")
            nc.vector.memset(scr[:], 0.0)
            ftr = sb.tile([32, 32], F32, name="ftr")
            score_sb = sb.tile([T, T], F32, name="score_sb")
            m_sb = sb.tile([T, 1], F32, name="m_sb")
            e_sb = sb.tile([T, T], F32, name="e_sb")
            ssum_sb = sb.tile([T, 1], F32, name="ssum_sb")
            lg_sb = sb.tile([T, 1], F32, name="lg_sb")
            fstage = sb.tile([T, U_CRF], F32, name="fstage")

            A_sb = sb.tile([T, T], F32, name="A_sb")
            nc.vector.tensor_copy(A_sb[:], eyelog)
            sc_row = sb.tile([1, 144], F32, name="sc_row")
            s1_sb = sb.tile([T, 144], F32, name="s1_sb")
            m2_sb = sb.tile([T, T], F32, name="m2_sb")
            e2_sb = sb.tile([T, 144], F32, name="e2_sb")
            e3_sb = sb.tile([T, 144], F32, name="e3_sb")
            ss2_sb = sb.tile([T, T], F32, name="ss2_sb")
            ln2_sb = sb.tile([T, T], F32, name="ln2_sb")

            def _bc3(ap2d, dims):
                return bass.AP(ap2d.tensor, ap2d.offset, [ap2d.ap[0]] + dims)

            _ftr0 = ftr[0:1, 0:12]
            frow_bc = _bc3(_ftr0, [[0, 12], [1, 12]])
            _A0 = A_sb[:]
            A_bc = _bc3(_A0, [[0, 12], [1, 12]])
            _m20 = m2_sb[:]
            m2_bc = _bc3(_m20, [[1, 12], [0, 12]])
            tkj3 = tkj_sb.rearrange("p (k j) -> p k j", j=12)
            sc3 = sc_row[:].rearrange("p (k j) -> p k j", j=12)
            s13 = s1_sb[:].rearrange("p (k j) -> p k j", j=12)
            e23 = e2_sb[:].rearrange("p (k j) -> p k j", j=12)
            e33 = e3_sb[:].rearrange("p (k j) -> p k j", j=12)
            m23 = m2_sb[:].rearrange("p (k j) -> p k j", j=1)
            ss23 = ss2_sb[:].rearrange("p (k j) -> p k j", j=1)

            def compose_step(u):
                nc.vector.tensor_copy(scr[0:T, 0:1], fstage[:, u:u + 1])
                nc.vector.transpose(ftr[:], scr[:])
                nc.vector.tensor_tensor(out=sc3, in0=tkj3, in1=frow_bc, op=OP.add)
                pb2 = ps.tile([T, 144], F32, name="pb2", tag="pb")
                nc.tensor.matmul(out=pb2[:], lhsT=ones12[0:1, :],
                                 rhs=sc_row[:], start=True, stop=True)
                nc.vector.tensor_tensor(
                    out=s13, in0=A_bc,
                    in1=pb2[:].rearrange("p (k j) -> p k j", j=12), op=OP.add)
                nc.vector.tensor_reduce(out=m23, in_=s13, axis=AX.X, op=OP.max,
                                        negate=True)
                nc.vector.tensor_tensor(out=e23, in0=s13, in1=m2_bc, op=OP.add)
                nc.scalar.activation(e3_sb[:], e2_sb[:], AF.Exp)
                nc.vector.tensor_reduce(out=ss23, in_=e33, axis=AX.X, op=OP.add)
                nc.scalar.activation(ln2_sb[:], ss2_sb[:], AF.Ln)
                nc.vector.tensor_tensor(out=A_sb[:], in0=ln2_sb[:], in1=m2_sb[:],
                                        op=OP.subtract)

            with tc.For_i(0, BLK // U_CRF) as it:
                nc.scalar.copy(
                    fstage[:],
                    f_blk[:, bass.ds((BLK - U_CRF) - U_CRF * it, U_CRF)])
                for u in range(U_CRF - 1, -1, -1):
                    compose_step(u)
            if debug:
                nc.sync.dma_start(adbg_d.ap(), A_sb[:])

            if upto <= 5:
                _trunc(A_sb[0:1, 0:1])
                return nc

            # ---------------- emit partial + AllGather payload [13, 12]
            oneh32 = sb.tile([T, BLK], F32, name="oneh32")
            nc.vector.tensor_copy(oneh32[:], oneh[:])
            dump_sb = sb.tile([T, BLK], F32, name="dump_sb")
            nc.vector.tensor_tensor(out=dump_sb[:], in0=f_blk[:], in1=oneh32[:],
                                    op=OP.mult)
            ev_sb = sb.tile([T, 1], F32, name="ev_sb")
            nc.vector.tensor_reduce(out=ev_sb[:], in_=dump_sb[:], axis=AX.X,
                                    op=OP.add)
            sel13 = sb.tile([T, 13], F32, name="sel13")
            nc.vector.memset(sel13[:], 0.0)
            nc.vector.memset(sel13[:, 12:13], 1.0)
            em_ps = ps.tile([13, 1], F32, name="em_ps", tag="pb")
            nc.tensor.matmul(out=em_ps[:], lhsT=sel13[:], rhs=ev_sb[:],
                             start=True, stop=True)

            pay = sb.tile([13, T], F32, name="pay")
            nc.vector.memset(pay[:], 0.0)
            nc.vector.tensor_copy(pay[0:T, :], A_sb[:])
            # em_ps rows 0..11 are zero, row 12 = emit partial: add into col 0
            nc.vector.tensor_tensor(out=pay[:, 0:1], in0=pay[:, 0:1],
                                    in1=em_ps[:], op=OP.add)

            cc2_in = dr.tile([13, T], F32, name="cc2_in")
            cc2_out = dr.tile([8 * 13, T], F32, name="cc2_out")
            nc.sync.dma_start(cc2_in[:], pay[:])
            nc.gpsimd.collective_compute(
                "AllGather", OP.bypass,
                replica_groups=[list(range(8))],
                ins=[cc2_in[:].opt()], outs=[cc2_out[:].opt()],
            )

            # ---------------- fold 8 blocks -> alpha
            def fold_step(mat_ap):
                pb = ps.tile([T, T], F32, name="pb", tag="pb")
                nc.tensor.matmul(out=pb[:], lhsT=ones12[0:1, :],
                                 rhs=prow[0:1, 0:T], start=True, stop=True)
                nc.vector.scalar_tensor_tensor(
                    out=score_sb[:], in0=mat_ap, scalar=0.0, in1=pb[:],
                    op0=OP.add, op1=OP.add)
                nc.vector.tensor_reduce(
                    out=m_sb[:], in_=score_sb[:], axis=AX.X, op=OP.max,
                    negate=True)
                nc.scalar.activation(e_sb[:], score_sb[:], AF.Exp,
                                     bias=m_sb[:, 0:1])
                nc.vector.tensor_reduce(
                    out=ssum_sb[:], in_=e_sb[:], axis=AX.X, op=OP.add)
                nc.scalar.activation(lg_sb[:], ssum_sb[:], AF.Ln)
                nc.vector.tensor_tensor(
                    out=scr[0:T, 0:1], in0=lg_sb[:], in1=m_sb[:], op=OP.subtract)
                nc.vector.transpose(prow[:], scr[:])

            for c in range(8):
                bct = sb.tile([T, T], F32, name="bct", tag="bct", bufs=2)
                nc.sync.dma_start(bct[:], cc2_out[:][13 * c:13 * c + 12, :])
                fold_step(bct[:])

            fin_sb = sb.tile([1, T], F32, name="fin_sb")
            nc.vector.tensor_tensor(out=fin_sb[:], in0=prow[0:1, 0:T],
                                    in1=tend_sb, op=OP.add)
            mf_sb = sb.tile([1, 1], F32, name="mf_sb")
            nc.vector.tensor_reduce(out=mf_sb[:], in_=fin_sb[:], axis=AX.X,
                                    op=OP.max, negate=True)
            ef_sb = sb.tile([1, T], F32, name="ef_sb")
            nc.scalar.activation(ef_sb[:], fin_sb[:], AF.Exp, bias=mf_sb[:, 0:1])
            sf_sb = sb.tile([1, 1], F32, name="sf_sb")
            nc.vector.tensor_reduce(out=sf_sb[:], in_=ef_sb[:], axis=AX.X,
                                    op=OP.add)
            lf_sb = sb.tile([1, 1], F32, name="lf_sb")
            nc.scalar.activation(lf_sb[:], sf_sb[:], AF.Ln)
            alpha_sb = sb.tile([1, 1], F32, name="alpha_sb")
            nc.vector.tensor_tensor(out=alpha_sb[:], in0=lf_sb[:], in1=mf_sb[:],
                                    op=OP.subtract)

            # total emit = sum of the 8 payload scalars
            em8 = sb.tile([8, 1], F32, name="em8")
            cc2 = cc2_out[:]
            em_ap = bass.AP(cc2.tensor, cc2.offset + 12 * T, [[13 * T, 8], [1, 1]])
            nc.sync.dma_start(em8[:], em_ap)
            ones8 = sb.tile([8, 1], F32, name="ones8")
            nc.vector.memset(ones8[:], 1.0)
            es_ps = ps.tile([1, 1], F32, name="es_ps", tag="pb2")
            nc.tensor.matmul(out=es_ps[:], lhsT=em8[:], rhs=ones8[:],
                             start=True, stop=True)

            res_sb = sb.tile([1, 1], F32, name="res_sb")
            nc.vector.tensor_tensor(out=res_sb[:], in0=alpha_sb[:],
                                    in1=es_ps[:], op=OP.subtract)
            nc.sync.dma_start(out_d.ap(), res_sb[:])

    split_multi_waits(nc)
    return nc


# ---------------------------------------------------------------- entry point

_CACHED_NC = None


def kernel(**inputs):
    """Full-input BiLSTM-CRF NLL on 8 NeuronCores; returns scalar np.float32."""
    global _CACHED_NC
    from concourse.bass_utils import run_bass_kernel_spmd
    if _CACHED_NC is None:
        _CACHED_NC = build(debug=0)
    in_maps, gold_trans = prep_all(inputs)
    res = run_bass_kernel_spmd(_CACHED_NC, in_maps, core_ids=list(range(8)))
    out = np.float32(res.results[0]["out"][0, 0] - gold_trans)
    return np.asarray(out)


# revision 4
# speedup vs baseline: 1.0474x; 1.0474x over previous
"""BiLSTM-CRF Trainium kernel, v2: 8-way chunked recurrence.

Sharding (8-core SPMD, one program, per-core data):
 - cores 0-3: forward LSTM over token chunks [512j-64, 512j+512) (64-step
   warm-up, first 512-token chunk gets the exact h0/c0 via a masked state
   reset between loop iterations 3 and 4); cores 4-7: the same on the
   host-reversed token stream (backward direction).
 - weights ship as per-core quarters (k-chunk of [wihT; whhT], fp16) and are
   reassembled on-device with a subgroup AllGather ([[0..3],[4..7]]).
 - each core computes its 512-token half-feats (its direction's w_out part),
   transposes and indirect-DMA-scatters them into a global [2048, 12] DRAM
   buffer at host-computed rows (this also undoes the backward reversal),
   then one AllReduce(+) makes full feats visible everywhere.
 - CRF: each core gathers its 256-token block, composes the block transfer
   matrix sequentially (log-semiring [12,12] steps), AllGather + 8-step fold
   -> alpha; per-block gold emit partials ride along in the AllGather.
 - host adds the transition part of the gold score (pure host data).

Inputs per core: pk16 [128, 6656] f16, pk32 [128, 280] f32, pki [128, 8] i32
(~1.8 MB/core vs 13.8 MB for the unchunked kernel).

Gate order in the permuted layouts: i, f, o, g  (G index 0..3),
pre/act column cc = G*4 + q, h component kappa(p,q) = q*128 + p.
"""
import numpy as np
import concourse.bass as bass
import concourse.mybir as mybir
import concourse.tile as tile
from concourse.masks import make_identity

F32 = mybir.dt.float32
F16 = mybir.dt.float16
I32 = mybir.dt.int32
AF = mybir.ActivationFunctionType
OP = mybir.AluOpType
AX = mybir.AxisListType

S, V, E, HD, T = 2048, 50257, 512, 512, 12
NG = 4 * HD          # 2048 gate rows per direction
P = 128
L = 512              # output tokens per core
W = 64               # warm-up steps
N = L + W            # 576 LSTM steps per core
U_LSTM = 16
NITER = N // U_LSTM  # 36
RESET_IT = W // U_LSTM  # 4
BLK = S // 8         # 256 CRF tokens per core
U_CRF = 16
NEG = -1e6

OG = [0, 1, 3, 2]    # our gate G=[i,f,o,g] -> original block [i,f,g,o]

# pk16 column map
C_WQ, C_XT, C_OH = 0, 4096, 4096 + 4 * N
PK16_W = C_OH + BLK
# pk32 column map
C_GB, C_HI, C_CI, C_WO, C_MSK = 0, 16, 20, 24, 72
C_TR, C_EYE, C_BO, C_TKJ = 108, 120, 132, 136
C_TE = C_TKJ + 144
PK32_W = C_TE + 12


def split_multi_waits(nc) -> int:
    """Walrus builds accept at most one sync-wait/update per instruction:
    split extras onto NoOps on the same engine."""
    n_split = 0
    for f in nc.m.functions:
        for bb in f.blocks:
            insts = bb.instructions
            out = []
            changed = False
            for inst in insts:
                si = inst.sync_info
                if si is None:
                    out.append(inst)
                    continue
                waits = list(si.on_wait)
                updates = list(si.on_update)
                if len(waits) <= 1 and len(updates) <= 1:
                    out.append(inst)
                    continue
                changed = True
                eng = inst.engine
                pre = []
                for w in waits[:-1]:
                    nop = mybir.InstNoOp(
                        name=nc.get_next_instruction_name(), ins=[], outs=[]
                    )
                    nop.engine = eng
                    nop.sync_info = mybir.SyncInfo(on_wait=[w], on_update=[])
                    pre.append(nop)
                    n_split += 1
                post = []
                for u in updates[1:]:
                    nop = mybir.InstNoOp(
                        name=nc.get_next_instruction_name(), ins=[], outs=[]
                    )
                    nop.engine = eng
                    nop.sync_info = mybir.SyncInfo(on_wait=[], on_update=[u])
                    post.append(nop)
                    n_split += 1
                inst.sync_info = mybir.SyncInfo(
                    on_wait=waits[-1:], on_update=updates[:1]
                )
                out.extend(pre)
                out.append(inst)
                out.extend(post)
            if changed:
                bb.instructions = out
    return n_split


# ---------------------------------------------------------------- host prep

def perm_rec():
    n = np.arange(NG)
    j, rem = n // 512, n % 512
    cc, i = rem // 32, rem % 32
    G, q = cc // 4, cc % 4
    og = np.array(OG)[G]
    return og * 512 + q * 128 + 32 * j + i


def perm_in():
    n = np.arange(NG)
    m, pp = n // 128, n % 128
    G, q = m // 4, m % 4
    og = np.array(OG)[G]
    return og * 512 + q * 128 + pp


def w_chunked_T(Wp):
    """[NG, 512] f32 -> [128, 4*NG]: out[p, k*NG+n] = Wp[n, k*128+p]."""
    return np.ascontiguousarray(
        Wp.reshape(NG, 4, 128).transpose(2, 1, 0).reshape(P, 4 * NG))


def part_major(a, cols):
    """[rows, 128*cols]-style: [n, D] with D=128*cols -> [128, cols*n]?
    Here: vec arrangement [D] -> [128, D//128] with out[p, q] = a[q*128+p]."""
    return np.ascontiguousarray(a.reshape(cols, 128).T)


_PREP_CACHE = {}


def prep_all(inputs):
    """Build the 8 per-core in_maps. Returns (in_maps, gold_trans_scalar)."""
    sent = np.asarray(inputs["sentence"]).astype(np.int64).reshape(-1)
    gold = np.asarray(inputs["gold_tags"]).astype(np.int64).reshape(-1)
    emb = np.asarray(inputs["emb"], np.float32)
    trans = np.asarray(inputs["transitions"], np.float32)
    w_out = np.asarray(inputs["w_out"], np.float32)
    b_out = np.asarray(inputs["b_out"], np.float32)
    h0 = np.asarray(inputs["h0"], np.float32)
    c0 = np.asarray(inputs["c0"], np.float32)

    pr, pi = perm_rec(), perm_in()
    # per-direction weight packs [128, 4*NG] fp16 (input-proj + recurrent)
    dirw = []
    for d, (wih, whh, b) in enumerate((
        (inputs["wih_f"], inputs["whh_f"], inputs["b_f"]),
        (inputs["wih_b"], inputs["whh_b"], inputs["b_b"]),
    )):
        wih = np.asarray(wih, np.float32)
        whh = np.asarray(whh, np.float32)
        b = np.asarray(b, np.float32)
        dirw.append(dict(
            wihT=w_chunked_T(wih[pi]).astype(np.float16),
            whhT=w_chunked_T(whh[pr]).astype(np.float16),
            ginb=np.ascontiguousarray(b[pi].reshape(16, 128).T),
            h0p=part_major(h0[d], 4),
            c0p=part_major(c0[d], 4),
            woutT=np.ascontiguousarray(
                w_out[:, d * 512:(d + 1) * 512].reshape(T, 4, 128)
                .transpose(2, 1, 0).reshape(P, 48)),
        ))

    x_f = emb[sent]                       # [S, E] f32
    x_b = x_f[::-1]

    # gold transition score (host; exact)
    tags = np.concatenate([[0], gold])
    gold_trans = float(
        trans[tags[1:], tags[:-1]].astype(np.float64).sum()
    ) + float(trans[1, tags[-1]])

    in_maps = []
    for core in range(8):
        d, j = core // 4, core % 4
        dw = dirw[d]
        x = x_f if d == 0 else x_b

        # x window [512j-64, 512j+512), clipped; partition-major fp16
        pos = np.clip(512 * j - W + np.arange(N), 0, S - 1)
        xs = x[pos]                                        # [N, 512]
        xT = np.ascontiguousarray(
            xs.reshape(N, 4, 128).transpose(2, 1, 0).reshape(P, 4 * N)
        ).astype(np.float16)

        pk16 = np.zeros((P, PK16_W), np.float16)
        pk16[:, C_WQ + 0 * NG: C_WQ + 1 * NG] = dw["wihT"][:, j * NG:(j + 1) * NG]
        pk16[:, C_WQ + 1 * NG: C_WQ + 2 * NG] = dw["whhT"][:, j * NG:(j + 1) * NG]
        pk16[:, C_XT: C_XT + 4 * N] = xT
        # gold one-hot for CRF block [256*core, 256*core+256)
        gb = gold[BLK * core: BLK * (core + 1)]
        oh = np.zeros((T, BLK), np.float16)
        oh[gb, np.arange(BLK)] = 1.0
        pk16[0:T, C_OH: C_OH + BLK] = oh

        pk32 = np.zeros((P, PK32_W), np.float32)
        pk32[:, C_GB: C_GB + 16] = dw["ginb"]
        if j == 0:
            pk32[:, C_HI: C_HI + 4] = dw["h0p"]
            pk32[:, C_CI: C_CI + 4] = dw["c0p"]
        pk32[:, C_WO: C_WO + 48] = dw["woutT"]
        msk = np.ones((P, NITER), np.float32)
        if j == 0:
            msk[:, RESET_IT] = 0.0
        pk32[:, C_MSK: C_MSK + NITER] = msk
        pk32[0:T, C_TR: C_TR + T] = trans
        pk32[0:T, C_EYE: C_EYE + T] = np.where(
            np.eye(T, dtype=bool), 0.0, NEG).astype(np.float32)
        pk32[0:T, C_BO: C_BO + 1] = b_out.reshape(T, 1)
        pk32[0:1, C_TKJ: C_TKJ + 144] = np.ascontiguousarray(
            trans.T).reshape(1, 144)
        pk32[0:1, C_TE: C_TE + 12] = trans[1:2, :]

        pki = np.zeros((P, 8), np.int32)
        # feats scatter rows: local output col 128*kk+p -> global token row
        for kk in range(4):
            t_local = 512 * j + 128 * kk + np.arange(128)
            g_row = t_local if d == 0 else (S - 1) - t_local
            pki[:, kk] = g_row
        # CRF block gather rows
        for t in range(2):
            pki[:, 4 + t] = BLK * core + 128 * t + np.arange(128)

        in_maps.append(dict(pk16=pk16, pk32=pk32, pki=pki))
    return in_maps, gold_trans


# ---------------------------------------------------------------- device code

def build(debug=0, upto=99):
    """upto: 1=unpack+wAG, 2=+gin, 3=+LSTM, 4=+feats/AllReduce, 5=+CRF
    compose, 99=full. Truncated builds end with a dummy out write."""
    nc = bass.Bass("TRN2", target_bir_lowering=False, debug=False,
                   num_devices=8)

    pk16 = nc.dram_tensor("pk16", [P, PK16_W], F16, kind="ExternalInput")
    pk32 = nc.dram_tensor("pk32", [P, PK32_W], F32, kind="ExternalInput")
    pki = nc.dram_tensor("pki", [P, 8], I32, kind="ExternalInput")
    out_d = nc.dram_tensor("out", [1, 1], F32, kind="ExternalOutput")
    if debug:
        hdbg_d = nc.dram_tensor("hdbg", [P, 4 * N], F16, kind="ExternalOutput")
        fdbg_d = nc.dram_tensor("fdbg", [S, T], F32, kind="ExternalOutput")
        bdbg_d = nc.dram_tensor("bdbg", [T, BLK], F32, kind="ExternalOutput")
        adbg_d = nc.dram_tensor("adbg", [T, T], F32, kind="ExternalOutput")

    with tile.TileContext(nc) as tc:
        with (
            tc.tile_pool(name="sb", bufs=1) as sb,
            tc.tile_pool(name="ps", bufs=1, space="PSUM") as ps,
            tc.tile_pool(name="dr", bufs=1, space="DRAM") as dr,
        ):
            # ---------------- unpack + weight AllGather
            pk32_sb = sb.tile([P, PK32_W], F32, name="pk32_sb")
            nc.sync.dma_start(pk32_sb[:], pk32.ap())
            pki_sb = sb.tile([P, 8], I32, name="pki_sb")
            nc.sync.dma_start(pki_sb[:], pki.ap())
            xt_h = sb.tile([P, 4 * N], F16, name="xt_h")
            nc.sync.dma_start(xt_h[:], pk16.ap()[:, C_XT: C_XT + 4 * N])
            oneh = sb.tile([T, BLK], F16, name="oneh")
            nc.sync.dma_start(oneh[:], pk16.ap()[0:T, C_OH: C_OH + BLK])

            wst = sb.tile([P, 4096], F16, name="wst")
            nc.sync.dma_start(wst[:], pk16.ap()[:, C_WQ: C_WQ + 4096])
            wag_in = dr.tile([P, 4096], F16, name="wag_in")
            wcat = dr.tile([4 * P, 4096], F16, name="wcat")
            nc.sync.dma_start(wag_in[:], wst[:])
            nc.gpsimd.collective_compute(
                "AllGather", OP.bypass,
                replica_groups=[[0, 1, 2, 3], [4, 5, 6, 7]],
                ins=[wag_in[:].opt()], outs=[wcat[:].opt()],
            )
            wih_h = sb.tile([P, 4 * NG], F16, name="wih_h")
            whh_h = sb.tile([P, 4 * NG], F16, name="whh_h")
            for k in range(4):
                nc.sync.dma_start(
                    wih_h[:, k * NG:(k + 1) * NG],
                    wcat[:][128 * k:128 * (k + 1), 0:NG])
                nc.sync.dma_start(
                    whh_h[:, k * NG:(k + 1) * NG],
                    wcat[:][128 * k:128 * (k + 1), NG:2 * NG])

            ident = sb.tile([P, P], F32, name="ident")
            make_identity(nc, ident[:])

            def _trunc(src_ap):
                t_ = sb.tile([1, 1], F32, name="trunc")
                nc.vector.tensor_copy(t_[:], src_ap)
                nc.sync.dma_start(out_d.ap(), t_[:])

            if upto <= 1:
                _trunc(wih_h[0:1, 0:1])
                return nc

            gin_b = pk32_sb[:, C_GB: C_GB + 16]
            hI = pk32_sb[:, C_HI: C_HI + 4]
            cI = pk32_sb[:, C_CI: C_CI + 4]
            msk = pk32_sb[:, C_MSK: C_MSK + NITER]
            trans_sb = pk32_sb[0:T, C_TR: C_TR + T]
            eyelog = pk32_sb[0:T, C_EYE: C_EYE + T]
            bout = pk32_sb[0:T, C_BO: C_BO + 1]
            tkj_sb = pk32_sb[0:1, C_TKJ: C_TKJ + 144]
            tend_sb = pk32_sb[0:1, C_TE: C_TE + 12]

            wout_h = sb.tile([P, 48], F16, name="wout_h")
            nc.vector.tensor_copy(wout_h[:], pk32_sb[:, C_WO: C_WO + 48])
            hI_h = sb.tile([P, 4], F16, name="hI_h")
            nc.vector.tensor_copy(hI_h[:], hI)

            # ---------------- input projection: gin[p, u*16+m]
            gin_sb = sb.tile([P, 16 * N], F16, name="gin_sb")
            gin_tm = gin_sb[:].rearrange("p (t m) -> p t m", m=16)
            pp_tag = dict(tag="pp", bufs=2)
            for m in range(16):
                for s0, sl in ((0, 512), (512, N - 512)):
                    pp = ps.tile([P, 512], F32, name="pp", **pp_tag)
                    for k in range(4):
                        nc.tensor.matmul(
                            out=pp[:, 0:sl],
                            lhsT=wih_h[:, k * NG + m * P: k * NG + (m + 1) * P],
                            rhs=xt_h[:, k * N + s0: k * N + s0 + sl],
                            start=(k == 0), stop=(k == 3),
                        )
                    nc.vector.tensor_tensor(
                        out=gin_tm[:, s0:s0 + sl, m:m + 1],
                        in0=pp[:, 0:sl].rearrange("p (t o) -> p t o", o=1),
                        in1=gin_b[:, m:m + 1].to_broadcast([P, sl]).rearrange(
                            "p (t o) -> p t o", o=1),
                        op=OP.add,
                    )

            if upto <= 2:
                _trunc(gin_sb[0:1, 0:1])
                return nc

            # ---------------- LSTM: N steps, masked state reset at it=4
            H_h = sb.tile([P, 4 * N + 4], F16, name="H_h")
            hstage = sb.tile([P, 4 * U_LSTM], F16, name="hstage")
            nc.vector.memset(hstage[:], 0.0)
            c_sb = sb.tile([P, 4], F32, name="c_sb")
            nc.vector.memset(c_sb[:], 0.0)
            g_ps = ps.tile([P, 512], F32, name="g_ps", tag="g")
            nc.vector.memset(g_ps[:], 0.0)
            gt_sb = sb.tile([P, 512], F32, name="gt_sb")
            pre_sb = sb.tile([P, 16], F32, name="pre_sb")
            act_sb = sb.tile([P, 16], F32, name="act_sb")
            z_sb = sb.tile([P, 4], F32, name="z_sb")
            fc_sb = sb.tile([P, 4], F32, name="fc_sb")
            tc_sb = sb.tile([P, 4], F32, name="tc_sb")
            th_sb = sb.tile([P, 4], F16, name="th_sb")
            tc4_sb = sb.tile([P, 4], F32, name="tc4_sb")
            gstage = sb.tile([P, 16 * U_LSTM], F16, name="gstage")

            _gt = gt_sb[:]
            gt_strided = bass.AP(_gt.tensor, _gt.offset, [_gt.ap[0], [32, 16]])

            def lstm_step(u):
                up = (u - 1) % U_LSTM
                for k in range(4):
                    lcol = hstage[:, 4 * up + k:4 * up + k + 1]
                    for jj in range(4):
                        nc.tensor.matmul(
                            out=g_ps[32 * jj:32 * jj + 1, :],
                            lhsT=lcol,
                            rhs=whh_h[:, k * NG + jj * 512: k * NG + (jj + 1) * 512],
                            start=(k == 0), stop=(k == 3),
                            tile_position=(0, 32 * jj),
                        )
                nc.vector.transpose(gt_sb[:], g_ps[:])
                nc.vector.tensor_tensor(
                    out=pre_sb[:], in0=gt_strided,
                    in1=gstage[:, 16 * u:16 * (u + 1)], op=OP.add,
                )
                nc.scalar.activation(act_sb[:, 0:12], pre_sb[:, 0:12], AF.Sigmoid)
                nc.scalar.activation(act_sb[:, 12:16], pre_sb[:, 12:16], AF.Tanh)
                nc.vector.tensor_tensor(
                    out=z_sb[:], in0=act_sb[:, 0:4], in1=act_sb[:, 12:16],
                    op=OP.mult)
                nc.vector.tensor_tensor(
                    out=fc_sb[:], in0=act_sb[:, 4:8], in1=c_sb[:], op=OP.mult)
                nc.vector.tensor_tensor(
                    out=c_sb[:], in0=fc_sb[:], in1=z_sb[:], op=OP.add)
                nc.scalar.activation(tc_sb[:], c_sb[:], AF.Tanh)
                nc.vector.tensor_tensor(
                    out=hstage[:, 4 * u:4 * (u + 1)], in0=act_sb[:, 8:12],
                    in1=tc_sb[:], op=OP.mult)

            with tc.For_i(0, NITER, hint_engines=(
                    mybir.EngineType.PE, mybir.EngineType.DVE,
                    mybir.EngineType.Activation)) as it:
                # masked state reset: h <- (h-hI)*m + hI, c likewise
                mcol = msk[:, bass.ds(it, 1)]
                nc.vector.tensor_tensor(
                    out=th_sb[:], in0=hstage[:, 60:64], in1=hI_h[:], op=OP.subtract)
                nc.vector.scalar_tensor_tensor(
                    out=hstage[:, 60:64], in0=th_sb[:], scalar=mcol,
                    in1=hI_h[:], op0=OP.mult, op1=OP.add)
                nc.vector.tensor_tensor(
                    out=tc4_sb[:], in0=c_sb[:], in1=cI, op=OP.subtract)
                nc.vector.scalar_tensor_tensor(
                    out=c_sb[:], in0=tc4_sb[:], scalar=mcol,
                    in1=cI, op0=OP.mult, op1=OP.add)
                nc.scalar.copy(gstage[:],
                               gin_sb[:, bass.ds(16 * U_LSTM * it, 16 * U_LSTM)])
                for u in range(U_LSTM):
                    lstm_step(u)
                nc.scalar.copy(H_h[:, bass.ds(4 * U_LSTM * it, 4 * U_LSTM)],
                               hstage[:])

            if debug:
                nc.sync.dma_start(hdbg_d.ap(), H_h[:, 0:4 * N])

            if upto <= 3:
                _trunc(H_h[0:1, 0:1])
                return nc

            # ---------------- feats: [12, 512] half-feats -> scatter -> AllReduce
            fp = ps.tile([T, 512], F32, name="fp", **pp_tag)
            Hwin = H_h[:, 4 * W: 4 * (W + L)].rearrange("p (t q) -> p t q", q=4)
            for q in range(4):
                nc.tensor.matmul(
                    out=fp[:], lhsT=wout_h[:, q * 12:(q + 1) * 12],
                    rhs=Hwin[:, :, q:q + 1], start=(q == 0), stop=(q == 3),
                )
            f_my = sb.tile([T, 512], F32, name="f_my")
            nc.vector.tensor_copy(f_my[:], fp[:])

            gfeats = dr.tile([S, T], F32, name="gfeats")
            zed = sb.tile([P, S * T // P], F32, name="zed")
            nc.vector.memset(zed[:], 0.0)
            nc.sync.dma_start(
                gfeats[:].rearrange("(p a) f -> p (a f)", p=P), zed[:])
            tp_tag = dict(tag="tp", bufs=2)
            for kk in range(4):
                tp = ps.tile([P, T], F32, name="tp", **tp_tag)
                nc.tensor.transpose(
                    out=tp[:], in_=f_my[:, P * kk: P * (kk + 1)],
                    identity=ident[0:T, 0:T])
                ft = sb.tile([P, T], F32, name="ft", tag="ft", bufs=2)
                nc.scalar.activation(ft[:], tp[:], AF.Copy)
                nc.gpsimd.indirect_dma_start(
                    out=gfeats[:], out_offset=bass.IndirectOffsetOnAxis(
                        ap=pki_sb[:, kk:kk + 1], axis=0),
                    in_=ft[:], in_offset=None,
                )
            gfall = dr.tile([S, T], F32, name="gfall")
            nc.gpsimd.collective_compute(
                "AllReduce", OP.add,
                replica_groups=[list(range(8))],
                ins=[gfeats[:].opt()], outs=[gfall[:].opt()],
            )
            if debug:
                nc.sync.dma_start(fdbg_d.ap(), gfall[:])

            # ---------------- CRF block: gather + compose
            f_blk = sb.tile([T, BLK], F32, name="f_blk")
            for t in range(2):
                fbp = sb.tile([P, T], F32, name="fbp", tag="ft", bufs=2)
                nc.gpsimd.indirect_dma_start(
                    out=fbp[:], out_offset=None, in_=gfall[:],
                    in_offset=bass.IndirectOffsetOnAxis(
                        ap=pki_sb[:, 4 + t:5 + t], axis=0),
                )
                tpc = ps.tile([T, P], F32, name="tpc", **tp_tag)
                nc.tensor.transpose(out=tpc[:], in_=fbp[:], identity=ident[:])
                nc.scalar.activation(
                    f_blk[:, P * t: P * (t + 1)], tpc[:], AF.Copy)
            nc.vector.tensor_scalar(
                out=f_blk[:], in0=f_blk[:], scalar1=bout[:, 0:1],
                scalar2=None, op0=OP.add)
            if debug:
                nc.sync.dma_start(bdbg_d.ap(), f_blk[:])

            if upto <= 4:
                _trunc(f_blk[0:1, 0:1])
                return nc

            # compose: A <- A o Step_u, descending u
            ones12 = sb.tile([1, T], F32, name="ones12")
            nc.vector.memset(ones12[:], 1.0)
            prow = sb.tile([32, 32], F32, name="prow")
            nc.vector.memset(prow[:], 0.0)
            nc.vector.memset(prow[0:1, 1:T], NEG)
            scr = sb.tile([32, 32], F32, name="scr")
            nc.vector.memset(scr[:], 0.0)
            ftr = sb.tile([32, 32], F32, name="ftr")
            score_sb = sb.tile([T, T], F32, name="score_sb")
            m_sb = sb.tile([T, 1], F32, name="m_sb")
            e_sb = sb.tile([T, T], F32, name="e_sb")
            ssum_sb = sb.tile([T, 1], F32, name="ssum_sb")
            lg_sb = sb.tile([T, 1], F32, name="lg_sb")
            fstage = sb.tile([T, U_CRF], F32, name="fstage")

            A_sb = sb.tile([T, T], F32, name="A_sb")
            nc.vector.tensor_copy(A_sb[:], eyelog)
            sc_row = sb.tile([1, 144], F32, name="sc_row")
            s1_sb = sb.tile([T, 144], F32, name="s1_sb")
            m2_sb = sb.tile([T, T], F32, name="m2_sb")
            e2_sb = sb.tile([T, 144], F32, name="e2_sb")
            e3_sb = sb.tile([T, 144], F32, name="e3_sb")
            ss2_sb = sb.tile([T, T], F32, name="ss2_sb")
            ln2_sb = sb.tile([T, T], F32, name="ln2_sb")

            def _bc3(ap2d, dims):
                return bass.AP(ap2d.tensor, ap2d.offset, [ap2d.ap[0]] + dims)

            _ftr0 = ftr[0:1, 0:12]
            frow_bc = _bc3(_ftr0, [[0, 12], [1, 12]])
            _A0 = A_sb[:]
            A_bc = _bc3(_A0, [[0, 12], [1, 12]])
            _m20 = m2_sb[:]
            m2_bc = _bc3(_m20, [[1, 12], [0, 12]])
            tkj3 = tkj_sb.rearrange("p (k j) -> p k j", j=12)
            sc3 = sc_row[:].rearrange("p (k j) -> p k j", j=12)
            s13 = s1_sb[:].rearrange("p (k j) -> p k j", j=12)
            e23 = e2_sb[:].rearrange("p (k j) -> p k j", j=12)
            e33 = e3_sb[:].rearrange("p (k j) -> p k j", j=12)
            m23 = m2_sb[:].rearrange("p (k j) -> p k j", j=1)
            ss23 = ss2_sb[:].rearrange("p (k j) -> p k j", j=1)

            def compose_step(u):
                nc.vector.tensor_copy(scr[0:T, 0:1], fstage[:, u:u + 1])
                nc.vector.transpose(ftr[:], scr[:])
                nc.vector.tensor_tensor(out=sc3, in0=tkj3, in1=frow_bc, op=OP.add)
                pb2 = ps.tile([T, 144], F32, name="pb2", tag="pb")
                nc.tensor.matmul(out=pb2[:], lhsT=ones12[0:1, :],
                                 rhs=sc_row[:], start=True, stop=True)
                nc.vector.tensor_tensor(
                    out=s13, in0=A_bc,
                    in1=pb2[:].rearrange("p (k j) -> p k j", j=12), op=OP.add)
                nc.vector.tensor_reduce(out=m23, in_=s13, axis=AX.X, op=OP.max,
                                        negate=True)
                nc.vector.tensor_tensor(out=e23, in0=s13, in1=m2_bc, op=OP.add)
                nc.scalar.activation(e3_sb[:], e2_sb[:], AF.Exp)
                nc.vector.tensor_reduce(out=ss23, in_=e33, axis=AX.X, op=OP.add)
                nc.scalar.activation(ln2_sb[:], ss2_sb[:], AF.Ln)
                nc.vector.tensor_tensor(out=A_sb[:], in0=ln2_sb[:], in1=m2_sb[:],
                                        op=OP.subtract)

            with tc.For_i(0, BLK // U_CRF) as it:
                nc.scalar.copy(
                    fstage[:],
                    f_blk[:, bass.ds((BLK - U_CRF) - U_CRF * it, U_CRF)])
                for u in range(U_CRF - 1, -1, -1):
                    compose_step(u)
            if debug:
                nc.sync.dma_start(adbg_d.ap(), A_sb[:])

            if upto <= 5:
                _trunc(A_sb[0:1, 0:1])
                return nc

            # ---------------- emit partial + AllGather payload [13, 12]
            oneh32 = sb.tile([T, BLK], F32, name="oneh32")
            nc.vector.tensor_copy(oneh32[:], oneh[:])
            dump_sb = sb.tile([T, BLK], F32, name="dump_sb")
            nc.vector.tensor_tensor(out=dump_sb[:], in0=f_blk[:], in1=oneh32[:],
                                    op=OP.mult)
            ev_sb = sb.tile([T, 1], F32, name="ev_sb")
            nc.vector.tensor_reduce(out=ev_sb[:], in_=dump_sb[:], axis=AX.X,
                                    op=OP.add)
            sel13 = sb.tile([T, 13], F32, name="sel13")
            nc.vector.memset(sel13[:], 0.0)
            nc.vector.memset(sel13[:, 12:13], 1.0)
            em_ps = ps.tile([13, 1], F32, name="em_ps", tag="pb")
            nc.tensor.matmul(out=em_ps[:], lhsT=sel13[:], rhs=ev_sb[:],
                             start=True, stop=True)

            pay = sb.tile([13, T], F32, name="pay")
            nc.vector.memset(pay[:], 0.0)
            nc.vector.tensor_copy(pay[0:T, :], A_sb[:])
            # em_ps rows 0..11 are zero, row 12 = emit partial: add into col 0
            nc.vector.tensor_tensor(out=pay[:, 0:1], in0=pay[:, 0:1],
                                    in1=em_ps[:], op=OP.add)

            cc2_in = dr.tile([13, T], F32, name="cc2_in")
            cc2_out = dr.tile([8 * 13, T], F32, name="cc2_out")
            nc.sync.dma_start(cc2_in[:], pay[:])
            nc.gpsimd.collective_compute(
                "AllGather", OP.bypass,
                replica_groups=[list(range(8))],
                ins=[cc2_in[:].opt()], outs=[cc2_out[:].opt()],
            )

            # ---------------- fold 8 blocks -> alpha
            def fold_step(mat_ap):
                pb = ps.tile([T, T], F32, name="pb", tag="pb")
                nc.tensor.matmul(out=pb[:], lhsT=ones12[0:1, :],
                                 rhs=prow[0:1, 0:T], start=True, stop=True)
                nc.vector.scalar_tensor_tensor(
                    out=score_sb[:], in0=mat_ap, scalar=0.0, in1=pb[:],
                    op0=OP.add, op1=OP.add)
                nc.vector.tensor_reduce(
                    out=m_sb[:], in_=score_sb[:], axis=AX.X, op=OP.max,
                    negate=True)
                nc.scalar.activation(e_sb[:], score_sb[:], AF.Exp,
                                     bias=m_sb[:, 0:1])
                nc.vector.tensor_reduce(
                    out=ssum_sb[:], in_=e_sb[:], axis=AX.X, op=OP.add)
                nc.scalar.activation(lg_sb[:], ssum_sb[:], AF.Ln)
                nc.vector.tensor_tensor(
                    out=scr[0:T, 0:1], in0=lg_sb[:], in1=m_sb[:], op=OP.subtract)
                nc.vector.transpose(prow[:], scr[:])

            for c in range(8):
                bct = sb.tile([T, T], F32, name="bct", tag="bct", bufs=2)
                nc.sync.dma_start(bct[:], cc2_out[:][13 * c:13 * c + 12, :])
                fold_step(bct[:])

            fin_sb = sb.tile([1, T], F32, name="fin_sb")
            nc.vector.tensor_tensor(out=fin_sb[:], in0=prow[0:1, 0:T],
                                    in1=tend_sb, op=OP.add)
            mf_sb = sb.tile([1, 1], F32, name="mf_sb")
            nc.vector.tensor_reduce(out=mf_sb[:], in_=fin_sb[:], axis=AX.X,
                                    op=OP.max, negate=True)
            ef_sb = sb.tile([1, T], F32, name="ef_sb")
            nc.scalar.activation(ef_sb[:], fin_sb[:], AF.Exp, bias=mf_sb[:, 0:1])
            sf_sb = sb.tile([1, 1], F32, name="sf_sb")
            nc.vector.tensor_reduce(out=sf_sb[:], in_=ef_sb[:], axis=AX.X,
                                    op=OP.add)
            lf_sb = sb.tile([1, 1], F32, name="lf_sb")
            nc.scalar.activation(lf_sb[:], sf_sb[:], AF.Ln)
            alpha_sb = sb.tile([1, 1], F32, name="alpha_sb")
            nc.vector.tensor_tensor(out=alpha_sb[:], in0=lf_sb[:], in1=mf_sb[:],
                                    op=OP.subtract)

            # total emit = sum of the 8 payload scalars
            em8 = sb.tile([8, 1], F32, name="em8")
            cc2 = cc2_out[:]
            em_ap = bass.AP(cc2.tensor, cc2.offset + 12 * T, [[13 * T, 8], [1, 1]])
            nc.sync.dma_start(em8[:], em_ap)
            ones8 = sb.tile([8, 1], F32, name="ones8")
            nc.vector.memset(ones8[:], 1.0)
            es_ps = ps.tile([1, 1], F32, name="es_ps", tag="pb2")
            nc.tensor.matmul(out=es_ps[:], lhsT=em8[:], rhs=ones8[:],
                             start=True, stop=True)

            res_sb = sb.tile([1, 1], F32, name="res_sb")
            nc.vector.tensor_tensor(out=res_sb[:], in0=alpha_sb[:],
                                    in1=es_ps[:], op=OP.subtract)
            nc.sync.dma_start(out_d.ap(), res_sb[:])

    split_multi_waits(nc)
    return nc


# ---------------------------------------------------------------- entry point

_CACHED_NC = None


def kernel(**inputs):
    """Full-input BiLSTM-CRF NLL on 8 NeuronCores; returns scalar np.float32."""
    global _CACHED_NC
    from concourse.bass_utils import run_bass_kernel_spmd
    if _CACHED_NC is None:
        _CACHED_NC = build(debug=0)
    in_maps, gold_trans = prep_all(inputs)
    res = run_bass_kernel_spmd(_CACHED_NC, in_maps, core_ids=list(range(8)))
    out = np.float32(res.results[0]["out"][0, 0] - gold_trans)
    return np.asarray(out)
